# revision 57
# baseline (speedup 1.0000x reference)
"""MultiHeadedAttention Trainium2 kernel (v3: mixed-batch slot pipeline).

Problem: B=2, T=2048, D=1024, H=16 heads (DK=64), fp32 in/out, padding mask
on keys. out = softmax(mask(QWq (KWk)^T / 8)) @ (VWv) @ Wo^T + biases.

Sharding (8 cores): core c owns head pair {2c, 2c+1} (128 projection
columns) for BOTH batches.  Each core computes its pair's attention for
batch 0 (full 16 key chunks) and batch 1 (only the key chunks with any
valid keys -- 12 of 16 under the reference mask), which balances the exp
work (the hard ScalarE floor) evenly across all 8 cores.  The host sums
8 partial output projections per batch (+ bo).

Structure (per core): a flat slot pipeline over (batch, query-block of
512, key-chunk).  Per slot: 2 concurrent score matmuls (row-tiled K=64 at
partitions 0/64, one per head), ONE exp activation N=1024 covering both
heads, and lagged attn@V matmuls.  PSUM = exactly 8 banks: score
ping-pong 2x[128,2,512]f32, o2 accumulator [65,2,512]f32, borrow slot
[128,1024]f32.  Projections, v-projections and output-projection jobs are
borrow-slot fills, with batch 1's projections filling the back half.
x/k/v activation buffers are reused across the two batches (batch 1's
DMAs are emitted after batch 0's consumers).  Output is DMA'd as bf16,
transposed [D, T]; the host transposes and accumulates in fp32.
"""

import numpy as np
import ml_dtypes

import concourse.bass as bass
import concourse.bacc as bacc
import concourse.tile as tile
from concourse import mybir
from concourse.bass_utils import run_bass_kernel_spmd

B, T, D, H = 2, 2048, 1024, 16
DK = D // H  # 64
NCORES = 8
KC = T // 128   # 16 key chunks
DCH = D // 128  # 8 contraction chunks
NQB = 4         # query blocks of 512
F32 = mybir.dt.float32
BF16 = mybir.dt.bfloat16

MASK_NEG = -30000.0


def build_program(with_bv: bool, vc1: int):
    """vc1 = number of key chunks with any valid key in batch 1."""
    nc = bacc.Bacc("TRN2")
    vc = (KC, vc1)

    xq_d = nc.declare_dram_parameter("xq", [2, DCH, 128, T], BF16,
                                     isOutput=False)
    xk_d = nc.declare_dram_parameter("xk", [2, DCH, 128, T], BF16,
                                     isOutput=False)
    xv_d = nc.declare_dram_parameter("xv", [2, DCH, 128, T], BF16,
                                     isOutput=False)
    wq_d = nc.declare_dram_parameter("wq", [128, DCH, 128], BF16,
                                     isOutput=False)
    wk_d = nc.declare_dram_parameter("wk", [128, DCH, 128], BF16,
                                     isOutput=False)
    wv_d = nc.declare_dram_parameter("wv", [128, DCH, 128], BF16,
                                     isOutput=False)
    wo_d = nc.declare_dram_parameter("wo", [128, D], BF16, isOutput=False)
    mask_d = nc.declare_dram_parameter("maskb", [128, 2, KC], F32,
                                       isOutput=False)
    bq_d = nc.declare_dram_parameter("bq", [128, 1], F32, isOutput=False)
    bk_d = nc.declare_dram_parameter("bk", [128, 1], F32, isOutput=False)
    bv_d = nc.declare_dram_parameter("bv", [64, 2], F32, isOutput=False)
    out_d = nc.declare_dram_parameter("out", [2, D, T], BF16, isOutput=True)

    EXPF = mybir.ActivationFunctionType.Exp

    # slot list: batch 0 fully, then batch 1's valid chunks
    slots = [(b, qb, kc) for b in (0, 1) for qb in range(NQB)
             for kc in range(vc[b])]
    NSLOT = len(slots)                      # 64 + 4*vc1
    unit_of = {}
    for t, (b, qb, kc) in enumerate(slots):
        unit_of[t] = 4 * b + qb

    with tile.TileContext(nc) as tc:
        with (
            tc.tile_pool(name="persist", bufs=1) as pp,
            tc.tile_pool(name="xbuf", bufs=1) as xp,
            tc.tile_pool(name="psum", bufs=1, space="PSUM") as psp,
            tc.tile_pool(name="expool", bufs=18) as exp_pool,
            tc.tile_pool(name="normp", bufs=1) as norm_pool,
            tc.tile_pool(name="outp", bufs=2) as out_pool,
        ):
            wq_sb = pp.tile([128, DCH, 128], BF16, tag="wq")
            wk_sb = pp.tile([128, DCH, 128], BF16, tag="wk")
            wv_sb = pp.tile([128, DCH, 128], BF16, tag="wv")
            wo_sb = pp.tile([128, D], BF16, tag="wo")
            mask_sb = pp.tile([128, 2, KC], F32, tag="mask")
            bq_sb = pp.tile([128, 1], F32, tag="bq")
            bk_sb = pp.tile([128, 1], F32, tag="bk")
            bv_sb = pp.tile([64, 2], F32, tag="bv")
            qT_sb = pp.tile([128, 2, T], BF16, tag="qT")
            kT_sb = pp.tile([128, 2, T], BF16, tag="kT")
            v_sb = pp.tile([128, 2, KC, 2, 66], BF16, tag="v")
            xh_sb = [pp.tile([128, T], BF16, tag=f"xh{b}", name=f"xh{b}")
                     for b in (0, 1)]
            nc.vector.memset(v_sb[:, :, :, :, 64:65], 1.0)

            # dummy exp to pull the ACT table load into the DMA-wait window
            dmy = pp.tile([128, 16], F32, tag="dmy")
            dmy2 = pp.tile([128, 16], BF16, tag="dmy2")
            nc.vector.memset(dmy[:], 0.0)
            nc.scalar.activation(dmy2[:], dmy[:], EXPF)

            def xtiles(tag):
                return [xp.tile([128, T], BF16, tag=f"{tag}{k}",
                                name=f"{tag}{k}") for k in range(DCH)]

            # batch-0 activation buffers (batch 1 reuses them later)
            xk_sb = xtiles("xk")
            xq_sb = xtiles("xq")
            xv_sb = xtiles("xv")

            def dma_x(dst, b, nm, t1=T):
                src = {"xq": xq_d, "xk": xk_d, "xv": xv_d}[nm]
                for k in range(DCH):
                    nc.sync.dma_start(out=dst[k][:, 0:t1],
                                      in_=src[b, k, :, 0:t1])

            # ---- batch-0 DMAs (issue order = rough priority) ----
            nc.sync.dma_start(out=wk_sb[:], in_=wk_d[:])
            nc.sync.dma_start(out=wq_sb[:], in_=wq_d[:])
            nc.sync.dma_start(out=mask_sb[:], in_=mask_d[:])
            nc.sync.dma_start(out=bk_sb[:], in_=bk_d[:])
            nc.sync.dma_start(out=bq_sb[:], in_=bq_d[:])
            for k in range(DCH):
                nc.sync.dma_start(out=xk_sb[k][:], in_=xk_d[0, k])
            for k in range(DCH):
                nc.sync.dma_start(out=xq_sb[k][:, 0:512],
                                  in_=xq_d[0, k, :, 0:512])
            nc.sync.dma_start(out=wv_sb[:], in_=wv_d[:])
            for k in range(DCH):
                nc.sync.dma_start(out=xq_sb[k][:, 512:T],
                                  in_=xq_d[0, k, :, 512:T])
            dma_x(xv_sb, 0, "xv")
            nc.sync.dma_start(out=wo_sb[:], in_=wo_d[:])
            nc.sync.dma_start(out=bv_sb[:], in_=bv_d[:])

            # ---- helpers ----
            def emit_proj(dst, w_sb, x_sb, c0, width, b_sb, tag):
                pst = psp.tile([128, 1024], F32, tag=tag,
                               bufs=1 if tag in ("br", "o2") else 2,
                               name="pst")
                for k in range(DCH):
                    for n in range(width // 512):
                        nc.tensor.matmul(
                            pst[:, n * 512:(n + 1) * 512],
                            w_sb[:, k, :],
                            x_sb[k][:, c0 + n * 512:c0 + (n + 1) * 512],
                            start=(k == 0), stop=(k == DCH - 1),
                            skip_group_check=True,
                        )
                nc.vector.tensor_scalar_add(dst[:], pst[:, 0:width],
                                            b_sb[:, 0:1])

            def emit_vproj(b, g):
                """v for key chunks 4g..4g+3 of batch b."""
                vps = psp.tile([128, 4, 2, 64], F32, tag="br", bufs=1,
                               name="vps")
                for t in range(4):
                    tcn = 4 * g + t
                    for k in range(DCH):
                        nc.tensor.matmul(
                            vps[:, t, :, :],
                            xv_sb[k][:, tcn * 128:(tcn + 1) * 128],
                            wv_sb[:, k, :],
                            start=(k == 0), stop=(k == DCH - 1),
                            skip_group_check=True,
                        )
                nc.vector.tensor_copy(v_sb[:, b, 4 * g:4 * g + 4, :, 0:64],
                                      vps[:])

            def emit_outproj(job, tail=False, tag="br"):
                """out[b]^T rows 256*dcg.. for token block qb (2 d-chunks)."""
                b, qb, dcg = job
                po = psp.tile([128, 2, 512], F32, tag=tag,
                              bufs=1 if tag in ("br", "o2") else 2, name="po")
                for d2 in range(2):
                    dc = 2 * dcg + d2
                    nc.tensor.matmul(
                        po[:, d2, :],
                        wo_sb[:, dc * 128:(dc + 1) * 128],
                        xh_sb[b][:, qb * 512:(qb + 1) * 512],
                        start=True, stop=True,
                        skip_group_check=True,
                    )
                ot = out_pool.tile([128, 2, 512], BF16, tag="ot")
                if tail and (qb + dcg) % 2 == 0:
                    nc.scalar.copy(ot[:], po[:])
                else:
                    nc.vector.tensor_copy(ot[:], po[:])
                for d2 in range(2):
                    dc = 2 * dcg + d2
                    nc.sync.dma_start(
                        out=out_d[b, dc * 128:(dc + 1) * 128,
                                  qb * 512:(qb + 1) * 512],
                        in_=ot[:, d2, :])

            def emit_norm(b, qb, o2):
                rd = norm_pool.tile([1, 2, 512], F32, tag="rd", name="rd")
                rc = norm_pool.tile([1, 2, 512], F32, tag="rc", name="rc")
                nc.vector.tensor_copy(rd[:], o2[64:65, :, :])
                nc.vector.reciprocal_approx_fast(rc[:], rd[:])
                rb = norm_pool.tile([64, 2, 512], F32, tag="rb", name="rb")
                nc.gpsimd.partition_broadcast(rb[:], rc[:])
                off = qb * 512
                dst = xh_sb[b][0:64, off:off + 512]
                nc.vector.tensor_mul(dst, o2[0:64, 0, :], rb[:, 0, :])
                if with_bv:
                    nc.vector.tensor_scalar_add(dst, dst, bv_sb[:, 0:1])
                tmp = norm_pool.tile([64, 512], BF16, tag="tmp", name="tmp",
                                     bufs=2)
                nc.vector.tensor_mul(tmp[:], o2[0:64, 1, :], rb[:, 1, :])
                if with_bv:
                    nc.vector.tensor_scalar_add(tmp[:], tmp[:], bv_sb[:, 1:2])
                nc.sync.dma_start(out=xh_sb[b][64:128, off:off + 512],
                                  in_=tmp[:])

            # ---- startup: kT(b0) full + qT(b0, qb0) ----
            pst0 = psp.tile([128, 1024], F32, tag="br", bufs=1, name="pst0")
            pst1 = psp.tile([128, 1024], F32, tag="sc", bufs=2, name="pst1")
            for k in range(DCH):
                for half, pst in ((0, pst0), (1, pst1)):
                    for n in range(2):
                        c = half * 1024 + n * 512
                        nc.tensor.matmul(
                            pst[:, n * 512:(n + 1) * 512],
                            wk_sb[:, k, :], xk_sb[k][:, c:c + 512],
                            start=(k == 0), stop=(k == DCH - 1),
                            skip_group_check=True,
                        )
            nc.vector.tensor_scalar_add(kT_sb[:, 0, 0:1024], pst0[:],
                                        bk_sb[:, 0:1])
            nc.vector.tensor_scalar_add(kT_sb[:, 0, 1024:2048], pst1[:],
                                        bk_sb[:, 0:1])
            emit_proj(qT_sb[:, 0, 0:512], wq_sb, xq_sb, 0, 512, bq_sb, "sc")

            # batch-1 x buffers: same tags -> reuse after b0 consumers
            xk1_sb = [None]
            xq1_sb = [None]
            xv1_sb = [None]

            def load_b1(nm, holder):
                holder[0] = xtiles(nm)
                # batch 1 only needs the valid key/value chunks; queries full
                t1 = T if nm == "xq" else 128 * vc1
                dma_x(holder[0], 1, nm, t1=t1)

            def emit_vproj_b1(g):
                vps = psp.tile([128, 4, 2, 64], F32, tag="br", bufs=1,
                               name="vps")
                ng = min(4, vc1 - 4 * g)
                for t in range(ng):
                    tcn = 4 * g + t
                    for k in range(DCH):
                        nc.tensor.matmul(
                            vps[:, t, :, :],
                            xv1_sb[0][k][:, tcn * 128:(tcn + 1) * 128],
                            wv_sb[:, k, :],
                            start=(k == 0), stop=(k == DCH - 1),
                            skip_group_check=True,
                        )
                nc.vector.tensor_copy(
                    v_sb[:, 1, 4 * g:4 * g + ng, :, 0:64], vps[:, 0:ng])

            # ---- borrow/fill plan ----
            plan = {}

            def at(s, fn, *a, **kw):
                plan.setdefault(s, []).append(lambda: fn(*a, **kw))

            at(3, emit_proj, qT_sb[:, 0, 512:1024], wq_sb, xq_sb, 512, 512,
               bq_sb, "br")
            at(6, load_b1, "xk", xk1_sb)           # after startup kT(b0)
            at(7, emit_proj, qT_sb[:, 0, 1024:1536], wq_sb, xq_sb, 1024, 512,
               bq_sb, "br")
            at(11, emit_proj, qT_sb[:, 0, 1536:2048], wq_sb, xq_sb, 1536,
               512, bq_sb, "br")
            at(14, load_b1, "xq", xq1_sb)          # reuses xq tags
            at(15, emit_vproj, 0, 0)
            at(19, emit_vproj, 0, 1)
            at(23, emit_vproj, 0, 2)
            at(27, emit_vproj, 0, 3)
            at(28, load_b1, "xv", xv1_sb)
            # batch-1 projections fill batch-0's back half (kT only over the
            # valid key range)
            kt1 = 128 * vc1
            at(32, lambda: emit_proj(kT_sb[:, 1, 0:min(1024, kt1)], wk_sb,
                                     xk1_sb[0], 0, min(1024, kt1), bk_sb,
                                     "br"))
            if kt1 > 1024:
                at(36, lambda: emit_proj(kT_sb[:, 1, 1024:kt1], wk_sb,
                                         xk1_sb[0], 1024, kt1 - 1024, bk_sb,
                                         "br"))
            at(42, lambda: emit_proj(qT_sb[:, 1, 0:512], wq_sb, xq1_sb[0],
                                     0, 512, bq_sb, "br"))
            at(48, lambda: emit_proj(qT_sb[:, 1, 512:1024], wq_sb, xq1_sb[0],
                                     512, 512, bq_sb, "br"))
            at(56, lambda: emit_proj(qT_sb[:, 1, 1024:1536], wq_sb, xq1_sb[0],
                                     1024, 512, bq_sb, "br"))
            at(70, lambda: emit_proj(qT_sb[:, 1, 1536:2048], wq_sb, xq1_sb[0],
                                     1536, 512, bq_sb, "br"))
            for s, g in ((52, 0), (60, 1), (64, 2), (66, 3)):
                if 4 * g < vc1:
                    at(s, emit_vproj_b1, g)

            vproj_slot = {(0, 0): 15, (0, 1): 19, (0, 2): 23, (0, 3): 27,
                          (1, 0): 52, (1, 1): 60, (1, 2): 64, (1, 3): 66}

            # ---- the slot loop ----
            ex_tiles = {}
            o2_cur = [None]
            vnext = [0]
            VLAG = 3
            norm_slot = {}
            out_jobs = [(b, qb, dcg) for b in (0, 1) for qb in range(NQB)
                        for dcg in range(4)]
            out_queue = [j for j in out_jobs if not (j[0] == 1 and j[1] == 3)]
            tail_only = [j for j in out_jobs if (j[0] == 1 and j[1] == 3)]
            out_min_slot = {}
            for j in out_jobs:
                b, qb, dcg = j
                base = (8 + 16 * qb + 20) if b == 0 else (64 + vc1 * qb + 18)
                out_min_slot[j] = base + 3 * dcg

            def emit_V(t, s):
                b, qb, kc = slots[t]
                u = unit_of[t]
                if kc == 0:
                    o2_cur[0] = psp.tile([65, 2, 512], F32, tag="o2", bufs=1,
                                         name="o2")
                o2 = o2_cur[0]
                for hh in range(2):
                    nc.tensor.matmul(
                        o2[:, hh, :],
                        v_sb[:, b, kc, hh, 0:65],
                        ex_tiles[t][:, hh, :],
                        start=(kc == 0), stop=(kc == vc[b] - 1),
                        skip_group_check=True,
                    )
                if kc == vc[b] - 1:
                    emit_norm(b, qb, o2)
                    norm_slot[u] = s
                del ex_tiles[t]

            def v_ready(t, s):
                if t > s - VLAG:
                    return False
                b, qb, kc = slots[t]
                if vproj_slot[(b, kc // 4)] + 5 > s:
                    return False
                return True

            for s in range(NSLOT):
                b, qb, kc = slots[s]
                sc = psp.tile([128, 2, 512], F32, tag="sc", bufs=2, name="sc")
                for hh in range(2):
                    nc.tensor.matmul(
                        sc[:, hh, :],
                        kT_sb[64 * hh:64 * hh + 64, b,
                              kc * 128:(kc + 1) * 128],
                        qT_sb[64 * hh:64 * hh + 64, b,
                              qb * 512:(qb + 1) * 512],
                        start=True, stop=True,
                    )
                ex = exp_pool.tile([128, 2, 512], BF16, tag="ex", name="ex")
                nc.scalar.activation(ex[:], sc[:], EXPF,
                                     bias=mask_sb[:, b, kc:kc + 1],
                                     scale=float(DK) ** -0.5)
                ex_tiles[s] = ex
                nv = 0
                while vnext[0] < NSLOT and nv < 3 and v_ready(vnext[0], s):
                    emit_V(vnext[0], s)
                    vnext[0] += 1
                    nv += 1
                has_plan = bool(plan.get(s))
                for fn in plan.get(s, []):
                    fn()
                if nv <= 1 and not has_plan:
                    # keep-warm: the PE HAM clock-gate re-throttles to
                    # 1.2GHz when array activity dips; burn a little PE on
                    # dummy matmuls in low-duty slots to hold 2.4GHz.
                    wt = psp.tile([128, 1024], F32, tag="br", bufs=1,
                                  name="wt")
                    for n in range(2):
                        nc.tensor.matmul(
                            wt[:, n * 512:(n + 1) * 512],
                            wk_sb[:, 0, :], wo_sb[:, 0:512],
                            start=True, stop=True, skip_group_check=True)
                while out_queue:
                    job = out_queue[0]
                    u = 4 * job[0] + job[1]
                    if (s >= out_min_slot[job]
                            and norm_slot.get(u, 9999) <= s - 3):
                        emit_outproj(out_queue.pop(0))
                    else:
                        break

            # ---- tail ----
            while vnext[0] < NSLOT:
                emit_V(vnext[0], NSLOT + 8)
                vnext[0] += 1
            tail_jobs = list(out_queue) + tail_only
            for i, job in enumerate(tail_jobs):
                emit_outproj(job, tail=True, tag=("br", "sc")[i % 2])

    nc.compile()
    return nc


_CACHE = {}


def _get_program(with_bv: bool, vc1: int):
    key = (with_bv, vc1)
    if key not in _CACHE:
        _CACHE[key] = build_program(with_bv, vc1)
    return _CACHE[key]


def make_in_maps(query, key, value, mask, Wq, bq, Wk, bk, Wv, bv, Wo, bo):
    bf = ml_dtypes.bfloat16
    xt = {}
    for nm, x in (("xq", query), ("xk", key), ("xv", value)):
        xt[nm] = np.stack([
            np.ascontiguousarray(x[b].T.reshape(DCH, 128, T)).astype(bf)
            for b in range(B)], 0)
    mb = np.where(np.asarray(mask)[:, 0] != 0, 0.0, MASK_NEG).astype(
        np.float32)  # [B, T]
    maskb = np.ascontiguousarray(
        mb.reshape(B, KC, 128).transpose(2, 0, 1))  # [128, 2, KC]
    in_maps = []
    for c in range(NCORES):
        cols = slice(128 * c, 128 * (c + 1))
        m = {"xq": xt["xq"], "xk": xt["xk"], "xv": xt["xv"], "maskb": maskb}
        for nm, W in (("wq", Wq), ("wk", Wk), ("wv", Wv)):
            m[nm] = np.ascontiguousarray(
                W[cols, :].T.reshape(DCH, 128, 128).transpose(1, 0, 2)
            ).astype(bf)
        m["wo"] = np.ascontiguousarray(Wo[:, cols].T).astype(bf)
        m["bq"] = np.ascontiguousarray(
            bq[cols].astype(np.float32).reshape(128, 1))
        m["bk"] = np.ascontiguousarray(
            bk[cols].astype(np.float32).reshape(128, 1))
        m["bv"] = np.ascontiguousarray(
            bv[cols].reshape(2, 64).T.astype(np.float32))
        in_maps.append(m)
    return in_maps


def kernel(query, key, value, mask, Wq, bq, Wk, bk, Wv, bv, Wo, bo,
           _trace=False):
    query, key, value = (np.asarray(a, np.float32) for a in (query, key, value))
    mask = np.asarray(mask)
    with_bv = bool(np.any(np.asarray(bv)))
    # number of key chunks with any valid key in batch 1
    m1 = mask[1, 0] != 0
    nz = np.nonzero(m1)[0]
    vc1 = int(nz.max() // 128 + 1) if len(nz) else 1
    nc = _get_program(with_bv, vc1)
    in_maps = make_in_maps(query, key, value, mask, Wq, bq, Wk, bk, Wv, bv,
                           Wo, bo)
    res = run_bass_kernel_spmd(nc, in_maps, list(range(NCORES)), trace=_trace)
    out = np.zeros((B, T, D), np.float32)
    for c in range(NCORES):
        o = np.asarray(res.results[c]["out"], np.float32)  # [2, D, T]
        for b in range(B):
            out[b] += o[b].T
    out += np.asarray(bo, np.float32)[None, None, :]
    if _trace:
        kernel.last_exec_time_ns = res.exec_time_ns
        kernel.last_results = res
    return out


# revision 58
# speedup vs baseline: 1.1774x; 1.1774x over previous
"""MultiHeadedAttention Trainium2 kernel (v3: mixed-batch slot pipeline).

Problem: B=2, T=2048, D=1024, H=16 heads (DK=64), fp32 in/out, padding mask
on keys. out = softmax(mask(QWq (KWk)^T / 8)) @ (VWv) @ Wo^T + biases.

Sharding (8 cores): core c owns head pair {2c, 2c+1} (128 projection
columns) for BOTH batches.  Each core computes its pair's attention for
batch 0 (full 16 key chunks) and batch 1 (only the key chunks with any
valid keys -- 12 of 16 under the reference mask), which balances the exp
work (the hard ScalarE floor) evenly across all 8 cores.  The host sums
8 partial output projections per batch (+ bo).

Structure (per core): a flat slot pipeline over (batch, query-block of
512, key-chunk).  Per slot: 2 concurrent score matmuls (row-tiled K=64 at
partitions 0/64, one per head), ONE exp activation N=1024 covering both
heads, and lagged attn@V matmuls.  PSUM = exactly 8 banks: score
ping-pong 2x[128,2,512]f32, o2 accumulator [65,2,512]f32, borrow slot
[128,1024]f32.  Projections, v-projections and output-projection jobs are
borrow-slot fills, with batch 1's projections filling the back half.
x/k/v activation buffers are reused across the two batches (batch 1's
DMAs are emitted after batch 0's consumers).  Output is DMA'd as bf16,
transposed [D, T]; the host transposes and accumulates in fp32.
"""

import numpy as np
import ml_dtypes

import concourse.bass as bass
import concourse.bacc as bacc
import concourse.tile as tile
from concourse import mybir
from concourse.bass_utils import run_bass_kernel_spmd

B, T, D, H = 2, 2048, 1024, 16
DK = D // H  # 64
NCORES = 8
KC = T // 128   # 16 key chunks
DCH = D // 128  # 8 contraction chunks
NQB = 4         # query blocks of 512
F32 = mybir.dt.float32
BF16 = mybir.dt.bfloat16

MASK_NEG = -30000.0


def build_program(with_bv: bool, vc1: int):
    """vc1 = number of key chunks with any valid key in batch 1."""
    nc = bacc.Bacc("TRN2")
    vc = (KC, vc1)

    xq_d = nc.declare_dram_parameter("xq", [2, DCH, 128, T], BF16,
                                     isOutput=False)
    xk_d = nc.declare_dram_parameter("xk", [2, DCH, 128, T], BF16,
                                     isOutput=False)
    xv_d = nc.declare_dram_parameter("xv", [2, DCH, 128, T], BF16,
                                     isOutput=False)
    wq_d = nc.declare_dram_parameter("wq", [128, DCH, 128], BF16,
                                     isOutput=False)
    wk_d = nc.declare_dram_parameter("wk", [128, DCH, 128], BF16,
                                     isOutput=False)
    wv_d = nc.declare_dram_parameter("wv", [128, DCH, 128], BF16,
                                     isOutput=False)
    wo_d = nc.declare_dram_parameter("wo", [128, D], BF16, isOutput=False)
    mask_d = nc.declare_dram_parameter("maskb", [128, 2, KC], F32,
                                       isOutput=False)
    bq_d = nc.declare_dram_parameter("bq", [128, 1], F32, isOutput=False)
    bk_d = nc.declare_dram_parameter("bk", [128, 1], F32, isOutput=False)
    bv_d = nc.declare_dram_parameter("bv", [64, 2], F32, isOutput=False)
    out_d = nc.declare_dram_parameter("out", [2, D, T], BF16, isOutput=True)

    EXPF = mybir.ActivationFunctionType.Exp

    # slot list: batch 0 fully, then batch 1's valid chunks
    slots = [(b, qb, kc) for b in (0, 1) for qb in range(NQB)
             for kc in range(vc[b])]
    NSLOT = len(slots)                      # 64 + 4*vc1
    unit_of = {}
    for t, (b, qb, kc) in enumerate(slots):
        unit_of[t] = 4 * b + qb

    with tile.TileContext(nc) as tc:
        with (
            tc.tile_pool(name="persist", bufs=1) as pp,
            tc.tile_pool(name="xbuf", bufs=1) as xp,
            tc.tile_pool(name="psum", bufs=1, space="PSUM") as psp,
            tc.tile_pool(name="expool", bufs=18) as exp_pool,
            tc.tile_pool(name="normp", bufs=1) as norm_pool,
            tc.tile_pool(name="outp", bufs=2) as out_pool,
        ):
            wq_sb = pp.tile([128, DCH, 128], BF16, tag="wq")
            wk_sb = pp.tile([128, DCH, 128], BF16, tag="wk")
            wv_sb = pp.tile([128, DCH, 128], BF16, tag="wv")
            wo_sb = pp.tile([128, D], BF16, tag="wo")
            mask_sb = pp.tile([128, 2, KC], F32, tag="mask")
            bq_sb = pp.tile([128, 1], F32, tag="bq")
            bk_sb = pp.tile([128, 1], F32, tag="bk")
            bv_sb = pp.tile([64, 2], F32, tag="bv")
            qT_sb = pp.tile([128, 2, T], BF16, tag="qT")
            kT_sb = pp.tile([128, 2, T], BF16, tag="kT")
            v_sb = pp.tile([128, 2, KC, 2, 66], BF16, tag="v")
            xh_sb = [pp.tile([128, T], BF16, tag=f"xh{b}", name=f"xh{b}")
                     for b in (0, 1)]
            nc.vector.memset(v_sb[:, :, :, :, 64:65], 1.0)

            # dummy exp to pull the ACT table load into the DMA-wait window
            dmy = pp.tile([128, 16], F32, tag="dmy")
            dmy2 = pp.tile([128, 16], BF16, tag="dmy2")
            nc.vector.memset(dmy[:], 0.0)
            nc.scalar.activation(dmy2[:], dmy[:], EXPF)

            def xtiles(tag):
                return [xp.tile([128, T], BF16, tag=f"{tag}{k}",
                                name=f"{tag}{k}") for k in range(DCH)]

            # batch-0 activation buffers (batch 1 reuses them later)
            xk_sb = xtiles("xk")
            xq_sb = xtiles("xq")
            xv_sb = xtiles("xv")

            def dma_x(dst, b, nm, t1=T):
                src = {"xq": xq_d, "xk": xk_d, "xv": xv_d}[nm]
                for k in range(DCH):
                    nc.sync.dma_start(out=dst[k][:, 0:t1],
                                      in_=src[b, k, :, 0:t1])

            # ---- batch-0 DMAs (issue order = rough priority) ----
            nc.sync.dma_start(out=wk_sb[:], in_=wk_d[:])
            nc.sync.dma_start(out=wq_sb[:], in_=wq_d[:])
            nc.sync.dma_start(out=mask_sb[:], in_=mask_d[:])
            nc.sync.dma_start(out=bk_sb[:], in_=bk_d[:])
            nc.sync.dma_start(out=bq_sb[:], in_=bq_d[:])
            for k in range(DCH):
                nc.sync.dma_start(out=xk_sb[k][:], in_=xk_d[0, k])
            for k in range(DCH):
                nc.sync.dma_start(out=xq_sb[k][:, 0:512],
                                  in_=xq_d[0, k, :, 0:512])
            nc.sync.dma_start(out=wv_sb[:], in_=wv_d[:])
            for k in range(DCH):
                nc.sync.dma_start(out=xq_sb[k][:, 512:T],
                                  in_=xq_d[0, k, :, 512:T])
            dma_x(xv_sb, 0, "xv")
            nc.sync.dma_start(out=wo_sb[:], in_=wo_d[:])
            nc.sync.dma_start(out=bv_sb[:], in_=bv_d[:])

            # ---- helpers ----
            def emit_proj(dst, w_sb, x_sb, c0, width, b_sb, tag):
                pst = psp.tile([128, 1024], F32, tag=tag,
                               bufs=1 if tag in ("br", "o2") else 2,
                               name="pst")
                for k in range(DCH):
                    for n in range(width // 512):
                        nc.tensor.matmul(
                            pst[:, n * 512:(n + 1) * 512],
                            w_sb[:, k, :],
                            x_sb[k][:, c0 + n * 512:c0 + (n + 1) * 512],
                            start=(k == 0), stop=(k == DCH - 1),
                            skip_group_check=True,
                        )
                nc.vector.tensor_scalar_add(dst[:], pst[:, 0:width],
                                            b_sb[:, 0:1])

            def emit_vproj(b, g):
                """v for key chunks 4g..4g+3 of batch b."""
                vps = psp.tile([128, 4, 2, 64], F32, tag="br", bufs=1,
                               name="vps")
                for t in range(4):
                    tcn = 4 * g + t
                    for k in range(DCH):
                        nc.tensor.matmul(
                            vps[:, t, :, :],
                            xv_sb[k][:, tcn * 128:(tcn + 1) * 128],
                            wv_sb[:, k, :],
                            start=(k == 0), stop=(k == DCH - 1),
                            skip_group_check=True,
                        )
                nc.vector.tensor_copy(v_sb[:, b, 4 * g:4 * g + 4, :, 0:64],
                                      vps[:])

            def emit_outproj(job, tail=False, tag="br"):
                """out[b]^T rows 256*dcg.. for token block qb (2 d-chunks)."""
                b, qb, dcg = job
                po = psp.tile([128, 2, 512], F32, tag=tag,
                              bufs=1 if tag in ("br", "o2") else 2, name="po")
                for d2 in range(2):
                    dc = 2 * dcg + d2
                    nc.tensor.matmul(
                        po[:, d2, :],
                        wo_sb[:, dc * 128:(dc + 1) * 128],
                        xh_sb[b][:, qb * 512:(qb + 1) * 512],
                        start=True, stop=True,
                        skip_group_check=True,
                    )
                ot = out_pool.tile([128, 2, 512], BF16, tag="ot")
                if tail and (qb + dcg) % 2 == 0:
                    nc.scalar.copy(ot[:], po[:])
                else:
                    nc.vector.tensor_copy(ot[:], po[:])
                for d2 in range(2):
                    dc = 2 * dcg + d2
                    nc.sync.dma_start(
                        out=out_d[b, dc * 128:(dc + 1) * 128,
                                  qb * 512:(qb + 1) * 512],
                        in_=ot[:, d2, :])

            def emit_norm(b, qb, o2):
                rd = norm_pool.tile([1, 2, 512], F32, tag="rd", name="rd")
                rc = norm_pool.tile([1, 2, 512], F32, tag="rc", name="rc")
                nc.vector.tensor_copy(rd[:], o2[64:65, :, :])
                nc.vector.reciprocal_approx_fast(rc[:], rd[:])
                rb = norm_pool.tile([64, 2, 512], F32, tag="rb", name="rb")
                nc.gpsimd.partition_broadcast(rb[:], rc[:])
                off = qb * 512
                dst = xh_sb[b][0:64, off:off + 512]
                nc.vector.tensor_mul(dst, o2[0:64, 0, :], rb[:, 0, :])
                if with_bv:
                    nc.vector.tensor_scalar_add(dst, dst, bv_sb[:, 0:1])
                tmp = norm_pool.tile([64, 512], BF16, tag="tmp", name="tmp",
                                     bufs=2)
                nc.vector.tensor_mul(tmp[:], o2[0:64, 1, :], rb[:, 1, :])
                if with_bv:
                    nc.vector.tensor_scalar_add(tmp[:], tmp[:], bv_sb[:, 1:2])
                nc.sync.dma_start(out=xh_sb[b][64:128, off:off + 512],
                                  in_=tmp[:])

            # ---- startup: kT(b0) full + qT(b0, qb0) ----
            pst0 = psp.tile([128, 1024], F32, tag="br", bufs=1, name="pst0")
            pst1 = psp.tile([128, 1024], F32, tag="sc", bufs=2, name="pst1")
            for k in range(DCH):
                for half, pst in ((0, pst0), (1, pst1)):
                    for n in range(2):
                        c = half * 1024 + n * 512
                        nc.tensor.matmul(
                            pst[:, n * 512:(n + 1) * 512],
                            wk_sb[:, k, :], xk_sb[k][:, c:c + 512],
                            start=(k == 0), stop=(k == DCH - 1),
                            skip_group_check=True,
                        )
            nc.vector.tensor_scalar_add(kT_sb[:, 0, 0:1024], pst0[:],
                                        bk_sb[:, 0:1])
            nc.vector.tensor_scalar_add(kT_sb[:, 0, 1024:2048], pst1[:],
                                        bk_sb[:, 0:1])
            emit_proj(qT_sb[:, 0, 0:512], wq_sb, xq_sb, 0, 512, bq_sb, "sc")

            # batch-1 x buffers: same tags -> reuse after b0 consumers
            xk1_sb = [None]
            xq1_sb = [None]
            xv1_sb = [None]

            def load_b1(nm, holder):
                holder[0] = xtiles(nm)
                # batch 1 only needs the valid key/value chunks; queries full
                t1 = T if nm == "xq" else 128 * vc1
                dma_x(holder[0], 1, nm, t1=t1)

            def emit_vproj_b1(g):
                vps = psp.tile([128, 4, 2, 64], F32, tag="br", bufs=1,
                               name="vps")
                ng = min(4, vc1 - 4 * g)
                for t in range(ng):
                    tcn = 4 * g + t
                    for k in range(DCH):
                        nc.tensor.matmul(
                            vps[:, t, :, :],
                            xv1_sb[0][k][:, tcn * 128:(tcn + 1) * 128],
                            wv_sb[:, k, :],
                            start=(k == 0), stop=(k == DCH - 1),
                            skip_group_check=True,
                        )
                nc.vector.tensor_copy(
                    v_sb[:, 1, 4 * g:4 * g + ng, :, 0:64], vps[:, 0:ng])

            # ---- borrow/fill plan ----
            plan = {}

            def at(s, fn, *a, **kw):
                plan.setdefault(s, []).append(lambda: fn(*a, **kw))

            at(3, emit_proj, qT_sb[:, 0, 512:1024], wq_sb, xq_sb, 512, 512,
               bq_sb, "br")
            at(6, load_b1, "xk", xk1_sb)           # after startup kT(b0)
            at(7, emit_proj, qT_sb[:, 0, 1024:1536], wq_sb, xq_sb, 1024, 512,
               bq_sb, "br")
            at(11, emit_proj, qT_sb[:, 0, 1536:2048], wq_sb, xq_sb, 1536,
               512, bq_sb, "br")
            at(14, load_b1, "xq", xq1_sb)          # reuses xq tags
            at(15, emit_vproj, 0, 0)
            at(19, emit_vproj, 0, 1)
            at(23, emit_vproj, 0, 2)
            at(27, emit_vproj, 0, 3)
            at(28, load_b1, "xv", xv1_sb)
            # batch-1 projections fill batch-0's back half (kT only over the
            # valid key range)
            kt1 = 128 * vc1
            at(32, lambda: emit_proj(kT_sb[:, 1, 0:min(1024, kt1)], wk_sb,
                                     xk1_sb[0], 0, min(1024, kt1), bk_sb,
                                     "br"))
            if kt1 > 1024:
                at(36, lambda: emit_proj(kT_sb[:, 1, 1024:kt1], wk_sb,
                                         xk1_sb[0], 1024, kt1 - 1024, bk_sb,
                                         "br"))
            at(42, lambda: emit_proj(qT_sb[:, 1, 0:512], wq_sb, xq1_sb[0],
                                     0, 512, bq_sb, "br"))
            at(48, lambda: emit_proj(qT_sb[:, 1, 512:1024], wq_sb, xq1_sb[0],
                                     512, 512, bq_sb, "br"))
            at(56, lambda: emit_proj(qT_sb[:, 1, 1024:1536], wq_sb, xq1_sb[0],
                                     1024, 512, bq_sb, "br"))
            at(70, lambda: emit_proj(qT_sb[:, 1, 1536:2048], wq_sb, xq1_sb[0],
                                     1536, 512, bq_sb, "br"))
            for s, g in ((52, 0), (60, 1), (64, 2), (66, 3)):
                if 4 * g < vc1:
                    at(s, emit_vproj_b1, g)

            vproj_slot = {(0, 0): 15, (0, 1): 19, (0, 2): 23, (0, 3): 27,
                          (1, 0): 52, (1, 1): 60, (1, 2): 64, (1, 3): 66}

            # ---- the slot loop ----
            ex_tiles = {}
            o2_cur = [None]
            vnext = [0]
            VLAG = 3
            norm_slot = {}
            out_jobs = [(b, qb, dcg) for b in (0, 1) for qb in range(NQB)
                        for dcg in range(4)]
            out_queue = [j for j in out_jobs if not (j[0] == 1 and j[1] == 3)]
            tail_only = [j for j in out_jobs if (j[0] == 1 and j[1] == 3)]
            out_min_slot = {}
            for j in out_jobs:
                b, qb, dcg = j
                base = (8 + 16 * qb + 20) if b == 0 else (64 + vc1 * qb + 18)
                out_min_slot[j] = base + 3 * dcg

            def emit_V(t, s):
                b, qb, kc = slots[t]
                u = unit_of[t]
                if kc == 0:
                    o2_cur[0] = psp.tile([65, 2, 512], F32, tag="o2", bufs=1,
                                         name="o2")
                o2 = o2_cur[0]
                for hh in range(2):
                    nc.tensor.matmul(
                        o2[:, hh, :],
                        v_sb[:, b, kc, hh, 0:65],
                        ex_tiles[t][:, hh, :],
                        start=(kc == 0), stop=(kc == vc[b] - 1),
                        skip_group_check=True,
                    )
                if kc == vc[b] - 1:
                    emit_norm(b, qb, o2)
                    norm_slot[u] = s
                del ex_tiles[t]

            def v_ready(t, s):
                if t > s - VLAG:
                    return False
                b, qb, kc = slots[t]
                if vproj_slot[(b, kc // 4)] + 5 > s:
                    return False
                return True

            for s in range(NSLOT):
                b, qb, kc = slots[s]
                sc = psp.tile([128, 2, 512], F32, tag="sc", bufs=2, name="sc")
                for hh in range(2):
                    nc.tensor.matmul(
                        sc[:, hh, :],
                        kT_sb[64 * hh:64 * hh + 64, b,
                              kc * 128:(kc + 1) * 128],
                        qT_sb[64 * hh:64 * hh + 64, b,
                              qb * 512:(qb + 1) * 512],
                        start=True, stop=True,
                    )
                ex = exp_pool.tile([128, 2, 512], BF16, tag="ex", name="ex")
                nc.scalar.activation(ex[:], sc[:], EXPF,
                                     bias=mask_sb[:, b, kc:kc + 1],
                                     scale=float(DK) ** -0.5)
                ex_tiles[s] = ex
                nv = 0
                while vnext[0] < NSLOT and nv < 3 and v_ready(vnext[0], s):
                    emit_V(vnext[0], s)
                    vnext[0] += 1
                    nv += 1
                for fn in plan.get(s, []):
                    fn()
                while out_queue:
                    job = out_queue[0]
                    u = 4 * job[0] + job[1]
                    if (s >= out_min_slot[job]
                            and norm_slot.get(u, 9999) <= s - 3):
                        emit_outproj(out_queue.pop(0))
                    else:
                        break

            # ---- tail ----
            while vnext[0] < NSLOT:
                emit_V(vnext[0], NSLOT + 8)
                vnext[0] += 1
            tail_jobs = list(out_queue) + tail_only
            for i, job in enumerate(tail_jobs):
                emit_outproj(job, tail=True, tag=("br", "sc")[i % 2])

    nc.compile()
    return nc


_CACHE = {}


def _get_program(with_bv: bool, vc1: int):
    key = (with_bv, vc1)
    if key not in _CACHE:
        _CACHE[key] = build_program(with_bv, vc1)
    return _CACHE[key]


def make_in_maps(query, key, value, mask, Wq, bq, Wk, bk, Wv, bv, Wo, bo):
    bf = ml_dtypes.bfloat16
    xt = {}
    for nm, x in (("xq", query), ("xk", key), ("xv", value)):
        xt[nm] = np.stack([
            np.ascontiguousarray(x[b].T.reshape(DCH, 128, T)).astype(bf)
            for b in range(B)], 0)
    mb = np.where(np.asarray(mask)[:, 0] != 0, 0.0, MASK_NEG).astype(
        np.float32)  # [B, T]
    maskb = np.ascontiguousarray(
        mb.reshape(B, KC, 128).transpose(2, 0, 1))  # [128, 2, KC]
    in_maps = []
    for c in range(NCORES):
        cols = slice(128 * c, 128 * (c + 1))
        m = {"xq": xt["xq"], "xk": xt["xk"], "xv": xt["xv"], "maskb": maskb}
        for nm, W in (("wq", Wq), ("wk", Wk), ("wv", Wv)):
            m[nm] = np.ascontiguousarray(
                W[cols, :].T.reshape(DCH, 128, 128).transpose(1, 0, 2)
            ).astype(bf)
        m["wo"] = np.ascontiguousarray(Wo[:, cols].T).astype(bf)
        m["bq"] = np.ascontiguousarray(
            bq[cols].astype(np.float32).reshape(128, 1))
        m["bk"] = np.ascontiguousarray(
            bk[cols].astype(np.float32).reshape(128, 1))
        m["bv"] = np.ascontiguousarray(
            bv[cols].reshape(2, 64).T.astype(np.float32))
        in_maps.append(m)
    return in_maps


def kernel(query, key, value, mask, Wq, bq, Wk, bk, Wv, bv, Wo, bo,
           _trace=False):
    query, key, value = (np.asarray(a, np.float32) for a in (query, key, value))
    mask = np.asarray(mask)
    with_bv = bool(np.any(np.asarray(bv)))
    # number of key chunks with any valid key in batch 1
    m1 = mask[1, 0] != 0
    nz = np.nonzero(m1)[0]
    vc1 = int(nz.max() // 128 + 1) if len(nz) else 1
    nc = _get_program(with_bv, vc1)
    in_maps = make_in_maps(query, key, value, mask, Wq, bq, Wk, bk, Wv, bv,
                           Wo, bo)
    res = run_bass_kernel_spmd(nc, in_maps, list(range(NCORES)), trace=_trace)
    out = np.zeros((B, T, D), np.float32)
    for c in range(NCORES):
        o = np.asarray(res.results[c]["out"], np.float32)  # [2, D, T]
        for b in range(B):
            out[b] += o[b].T
    out += np.asarray(bo, np.float32)[None, None, :]
    if _trace:
        kernel.last_exec_time_ns = res.exec_time_ns
        kernel.last_results = res
    return out
